# revision 20
# baseline (speedup 1.0000x reference)
"""AttentionConv Trainium2 kernel (8 NeuronCores, data-parallel over batch).

Reference math (per batch b, channel o, position (h,w), 7x7 window d=(di,dj)):
    q = wq @ x, k = wk @ x, v = wv @ x   (1x1 convs, channel matmuls)
    logits_d = q * k_d + rel             (k_d = zero-padded k shifted by d)
    out = sum_d softmax_d(logits) * v_d
`rel` is constant along the softmax axis, so it cancels. With zero padding,
out-of-bounds taps contribute exp(0)=1 to the denominator and 0 to the
numerator, which the zero-padded k/v slabs reproduce exactly.

v3 schedule (HW-measured: DVE bf16 tensor_tensor ~0.53ns/elem in 2x mode,
+3% for odd-offset windows; ACT 1/1.2GHz/elem; PE warm MM cadence 216ns at
N=512 with per-MM LDWEIGHTS overlapped by the reorder window):
  7 dj-groups; per group ONE mega t-mul t7[7,32,64] = bcast(q) * kwin4D
  (stride-0 q + overlapping [72,72,1] window AP), ONE 14336-elem exp, ONE
  mega u-mul, then 56 identity matmuls accumulating den/num in PSUM.
  DVE paces at ~15.3us/group; exp (12.2) and PE (12.1) hide under it.
  Tail: DVE reciprocal_approx_fast (saves the ACT ln/exp table + passes).
Partition layout: p = g*64 + o for H-halves g in {0,1}; free dim = (32,64).
GPSIMD only does head border memsets (its SBUF port contends with DVE).
"""

import numpy as np
import ml_dtypes

import concourse.bass as bass
import concourse.tile as tile
from concourse import bacc, mybir
from concourse.bass_utils import run_bass_kernel_spmd

N_CORES = 8
B, C, H, W, O = 8, 64, 64, 64, 64
KS, PAD = 7, 3
HW = H * W                      # 4096
HG = H // 2                     # 32 rows per partition group
NHALF = HG * W                  # 2048 free elements per partition
RSLAB = HG + KS - 1             # 38 padded slab rows
LPAD = 4                        # left col pad
CSLAB = W + LPAD + PAD + 1      # 72 cols (even row stride)
SLAB = RSLAB * CSLAB            # 2736

F32 = mybir.dt.float32
BF16 = mybir.dt.bfloat16
_NPBF16 = ml_dtypes.bfloat16

CFG = {
    "tbufs": 2, "ebufs": 2, "ubufs": 2,
    "t_split": 2,                # t-mul mega-ops per group
    "u_split": 2,                # u-mul mega-ops per group (earlier PE start)
    "u_split_last": 4,           # quarters on the last group: shorter PE drain
    "exp_split": 2,              # ACTIVATEs per group
    # odd dj first: those windows are 4B-aligned in the base slabs, so the
    # 1-shifted twins (for even dj) can be built on ACT slack meanwhile
    "dj_order": (1, 3, 5, 0, 2, 4, 6),
}


def build_program():
    nc = bacc.Bacc("TRN2", target_bir_lowering=False, debug=False,
                   num_devices=N_CORES)

    x_d = nc.dram_tensor("x", [C, HW], BF16, kind="ExternalInput").ap()
    wqT_d = nc.dram_tensor("wqT", [C, O], BF16, kind="ExternalInput").ap()
    wkT_d = nc.dram_tensor("wkT", [C, O], BF16, kind="ExternalInput").ap()
    wvT_d = nc.dram_tensor("wvT", [C, O], BF16, kind="ExternalInput").ap()
    ident_d = nc.dram_tensor("ident", [128, 128], BF16, kind="ExternalInput").ap()
    out_d = nc.dram_tensor("out", [O, H, W], BF16, kind="ExternalOutput").ap()

    with tile.TileContext(nc) as tc:
        _build(tc, x_d, wqT_d, wkT_d, wvT_d, ident_d, out_d)

    nc.compile()
    return nc


def _win(slab_flat, c0, di0, ndi):
    """[128, ndi, HG, W] overlapping window view of a [128, SLAB] slab:
    dims (di, r, c) with steps (CSLAB, CSLAB, 1), rows di0.., column c0."""
    t3 = slab_flat.rearrange("p (r c) -> p r c", r=RSLAB)
    v = t3[:, di0:di0 + HG, c0:c0 + W].unsqueeze(1).broadcast_to(
        [128, ndi, HG, W])
    w = v.copy()
    a = w.ap
    a[1] = [CSLAB, ndi]
    w.ap = a
    return w


def _build(tc, x_d, wqT_d, wkT_d, wvT_d, ident_d, out_d):
    nc = tc.nc
    from contextlib import ExitStack

    with ExitStack() as ctx:
        konst = ctx.enter_context(tc.tile_pool(name="konst", bufs=1))
        big = ctx.enter_context(tc.tile_pool(name="big", bufs=1))

        # --- inputs to SBUF ---
        # DMA issues serialize ~2.5us each per queue: wkT leads the sync
        # queue (tiny, unblocks PE warmup), then x ships as one half per
        # HWDGE queue into the same xb tile. ident rides Pool's SWDGE
        # after the memsets (first needed at the accum matmuls ~16us in).
        # xb lives in the long-lived pool: reusing its SBUF for the loop
        # pools would make the first t-mul wait on the v projection's last
        # xb read (measured +2.3us).
        wkT_sb = konst.tile([C, O], BF16, name="wkT_sb")
        nc.sync.dma_start(wkT_sb[:], wkT_d[:])
        xb = big.tile([C, HW], BF16, name="xb")
        # split at image row 40: the first part covers everything the first
        # k-projection chunk reads (rows 0-8 and 29-37), so it starts ~1us
        # before the full x lands; the tiny weights ride the scalar queue
        nc.sync.dma_start(xb[:, 0:40 * W], x_d[:, 0:40 * W])
        nc.sync.dma_start(xb[:, 40 * W:HW], x_d[:, 40 * W:HW])
        wqT_sb = konst.tile([C, O], BF16, name="wqT_sb")
        nc.scalar.dma_start(wqT_sb[:], wqT_d[:])
        ident_sb = konst.tile([128, 128], BF16, name="ident_sb")
        wvT_sb = konst.tile([C, O], BF16, name="wvT_sb")
        nc.scalar.dma_start(wvT_sb[:], wvT_d[:])

        # --- padded k/v slabs (zeroed borders) + q ---
        # (no shifted twins: their SBUF space is what xb now keeps; the
        # odd-offset window penalty is ~3% on the 4 even-dj groups)
        q_sb = big.tile([128, HG, W], BF16, name="q_sb")
        kp = big.tile([128, SLAB], BF16, name="kp")
        vp = big.tile([128, SLAB], BF16, name="vp")
        kp3 = kp.rearrange("p (r c) -> p r c", r=RSLAB)
        vp3 = vp.rearrange("p (r c) -> p r c", r=RSLAB)
        # GPSIMD memsets run first (no deps) — off DVE's port before the loop
        for t3 in (kp3, vp3):
            nc.gpsimd.memset(t3[:, 0:PAD, :], 0.0)
            nc.gpsimd.memset(t3[:, RSLAB - PAD:RSLAB, :], 0.0)
            nc.gpsimd.memset(t3[:, PAD:RSLAB - PAD, 0:LPAD], 0.0)
            nc.gpsimd.memset(t3[:, PAD:RSLAB - PAD, LPAD + W:CSLAB], 0.0)
        nc.gpsimd.dma_start(ident_sb[:], ident_d[:])

        proj_ctx = ExitStack()
        # bufs=6: with 4, the q-projection matmuls stalled ~1.5us on the
        # psum buffer rotation behind the k evacuations (measured)
        psum = proj_ctx.enter_context(
            tc.tile_pool(name="psum", bufs=6, space="PSUM"))

        # PE p-state warmup: dummy wkT@wkT matmuls keep PE busy from wkT
        # arrival (~9.3us) until x lands (~12.5us), so the projections run
        # at full clock (2.4GHz needs ~3us of continuous PE activity).
        warm = psum.tile([128, 512], F32, tag="proj", name="warm_ps")
        for wi in range(24):
            nc.tensor.matmul(warm[0:64, 0:64], wkT_sb[:], wkT_sb[:],
                             start=True, stop=True)

        # group 0 slab rows r hold image rows r-3 (valid r in [3,38));
        # group 1 slab rows r hold image rows r+29 (valid r in [0,35)).
        chunk_rows = [(0, 8), (8, 8), (16, 8), (24, 8), (32, 3)]

        def project_kv(wT_sb, dst3, name, evac_even, evac_odd):
            for ci, (r0, nr) in enumerate(chunk_rows):
                n = nr * W
                ps = psum.tile([128, 512], F32, tag="proj", name=f"{name}_ps{ci}")
                nc.tensor.matmul(ps[0:64, :n], wT_sb[:],
                                 xb[:, r0 * W:(r0 + nr) * W],
                                 start=True, stop=True)
                nc.tensor.matmul(ps[64:128, :n], wT_sb[:],
                                 xb[:, (29 + r0) * W:(29 + r0 + nr) * W],
                                 start=True, stop=True)
                src = ps[:, :n].rearrange("p (a b) -> p a b", a=nr)
                evac = evac_even if ci % 2 == 0 else evac_odd
                evac(dst3[0:64, 3 + r0:3 + r0 + nr, LPAD:LPAD + W], src[0:64])
                evac(dst3[64:128, r0:r0 + nr, LPAD:LPAD + W], src[64:128])

        # k first (the first t-mul needs it); evacs split DVE/ACT
        project_kv(wkT_sb, kp3, "k", nc.vector.tensor_copy, nc.scalar.copy)

        # q evacs alternate DVE/ACT: they gate the first t-mul
        for cchunk in range(4):
            ps = psum.tile([128, 512], F32, tag="proj", name=f"q_ps{cchunk}")
            for g in (0, 1):
                rhs = xb[:, g * NHALF + cchunk * 512: g * NHALF + (cchunk + 1) * 512]
                nc.tensor.matmul(ps[g * 64:(g + 1) * 64, :], wqT_sb[:], rhs,
                                 start=True, stop=True)
            evac = nc.vector.tensor_copy if cchunk % 2 == 0 else nc.scalar.copy
            evac(q_sb[:, cchunk * 8:(cchunk + 1) * 8, :],
                 ps[:].rearrange("p (a b) -> p a b", a=8))

        project_kv(wvT_sb, vp3, "v", nc.scalar.copy, nc.scalar.copy)

        proj_ctx.close()

        # --- main loop: 7 dj-groups x 7 di ---
        acc = ctx.enter_context(tc.tile_pool(name="acc", bufs=1, space="PSUM"))
        den_ps = acc.tile([128, NHALF], F32, name="den_ps")
        num_ps = acc.tile([128, NHALF], F32, name="num_ps")

        loop_ctx = ExitStack()
        tpool = loop_ctx.enter_context(tc.tile_pool(name="tpool", bufs=CFG["tbufs"]))
        epool = loop_ctx.enter_context(tc.tile_pool(name="epool", bufs=CFG["ebufs"]))
        upool = loop_ctx.enter_context(tc.tile_pool(name="upool", bufs=CFG["ubufs"]))

        qb = q_sb[:].unsqueeze(1).broadcast_to([128, KS, HG, W])

        n_off = KS * KS
        for gi, dj in enumerate(CFG["dj_order"]):
            c0 = dj + LPAD - PAD
            kb, kc = kp, c0
            vb, vc = vp, c0
            # last group: fine-grained interleaved t/exp/u pieces shorten
            # the end drain (the t->exp->u chain of the final di slices);
            # measured ~3us of DVE idle there with the coarse 2/2/4 splits
            last_group = gi == KS - 1
            tb = (0, 2, 4, 6, 7) if last_group else (0, 3, 7)
            eb = tb
            ub = (0, 2, 4, 6, 7) if last_group else (0, 3, 7)
            t7 = tpool.tile([128, KS, HG, W], BF16, tag="t", name=f"t7_{gi}")
            for si in range(len(tb) - 1):
                lo, hi = tb[si], tb[si + 1]
                nc.vector.tensor_mul(t7[:, lo:hi], qb[:, lo:hi],
                                     _win(kb, kc, lo, hi - lo))
            e7 = epool.tile([128, KS, HG, W], BF16, tag="e", name=f"e7_{gi}")
            t7f = t7.rearrange("p a b c -> p (a b c)")
            e7f = e7.rearrange("p a b c -> p (a b c)")
            for si in range(len(eb) - 1):
                lo, hi = eb[si] * NHALF, eb[si + 1] * NHALF
                nc.scalar.activation(e7f[:, lo:hi], t7f[:, lo:hi],
                                     mybir.ActivationFunctionType.Exp)
            u7 = upool.tile([128, KS, HG, W], BF16, tag="u", name=f"u7_{gi}")
            for si in range(len(ub) - 1):
                lo, hi = ub[si], ub[si + 1]
                nc.vector.tensor_mul(u7[:, lo:hi], e7[:, lo:hi],
                                     _win(vb, vc, lo, hi - lo))
            def den_mm(di, cc, start=False, stop=False):
                nc.tensor.matmul(
                    den_ps[:, cc * 512:(cc + 1) * 512], ident_sb[:],
                    e7[:, di, cc * 8:(cc + 1) * 8, :],
                    start=start, stop=stop, skip_group_check=True)

            def num_mm(di, cc, start=False, stop=False):
                nc.tensor.matmul(
                    num_ps[:, cc * 512:(cc + 1) * 512], ident_sb[:],
                    u7[:, di, cc * 8:(cc + 1) * 8, :],
                    start=start, stop=stop, skip_group_check=True)

            if not last_group:
                for di in range(KS):
                    first = gi == 0 and di == 0
                    for cc in range(4):
                        den_mm(di, cc, start=first)
                        num_mm(di, cc, start=first)
            else:
                # last group: all den matmuls first (they need only e7, so
                # they don't queue behind num matmuls waiting on u7 pieces),
                # chunk-major so the tail's recip of chunk 0 fires early
                for cc in range(4):
                    for di in range(KS):
                        den_mm(di, cc, stop=(cc == 3 and di == KS - 1))
                for di in range(KS):
                    for cc in range(4):
                        num_mm(di, cc, stop=(di == KS - 1 and cc == 3))

        loop_ctx.close()

        # --- divide and store (DVE approx reciprocal; den>0, well-scaled) ---
        tail_pool = ctx.enter_context(tc.tile_pool(name="tail", bufs=1))
        den_r = tail_pool.tile([128, NHALF], F32, name="den_r")
        out_sb = tail_pool.tile([128, NHALF], BF16, name="out_sb")
        out3 = out_sb.rearrange("p (a b) -> p a b", a=HG)
        for cc in range(4):
            sl = slice(cc * 512, (cc + 1) * 512)
            nc.vector.reciprocal_approx_fast(den_r[:, sl], den_ps[:, sl])
            nc.vector.tensor_mul(out_sb[:, sl], num_ps[:, sl], den_r[:, sl])
            rsl = slice(cc * 8, (cc + 1) * 8)
            q = nc.sync if cc % 2 == 0 else nc.scalar
            q.dma_start(out_d[:, rsl, :], out3[0:64, rsl, :])
            q.dma_start(out_d[:, HG + cc * 8:HG + (cc + 1) * 8, :],
                        out3[64:128, rsl, :])


_NC_CACHE = None


def _get_nc():
    global _NC_CACHE
    if _NC_CACHE is None:
        _NC_CACHE = build_program()
    return _NC_CACHE


def prepare_in_maps(x, wq, wk, wv):
    x = np.ascontiguousarray(np.asarray(x, np.float32).astype(_NPBF16))
    wqT = np.ascontiguousarray(np.asarray(wq, np.float32).T.astype(_NPBF16))
    wkT = np.ascontiguousarray(np.asarray(wk, np.float32).T.astype(_NPBF16))
    wvT = np.ascontiguousarray(np.asarray(wv, np.float32).T.astype(_NPBF16))
    ident = np.eye(128, dtype=_NPBF16)
    return [
        {"x": x[i].reshape(C, HW), "wqT": wqT, "wkT": wkT, "wvT": wvT,
         "ident": ident}
        for i in range(x.shape[0])
    ]


def run(in_maps, **kw):
    nc = _get_nc()
    return run_bass_kernel_spmd(nc, in_maps, list(range(N_CORES)), **kw)


def kernel(x, wq, wk, wv, rel_w=None, rel_h=None, kernel_size=7, padding=3,
           **_ignored):
    # rel_w/rel_h are constant along the softmax axis, so they cancel.
    assert int(kernel_size) == KS and int(padding) == PAD
    res = run(prepare_in_maps(x, wq, wk, wv))
    out = np.stack([res.results[i]["out"] for i in range(N_CORES)], axis=0)
    return out.astype(np.float32)


if __name__ == "__main__":
    rng = np.random.default_rng(0)
    x = rng.standard_normal((B, C, H, W), dtype=np.float32)
    wq = (rng.standard_normal((O, C)) * 0.1).astype(np.float32)
    wk = (rng.standard_normal((O, C)) * 0.1).astype(np.float32)
    wv = (rng.standard_normal((O, C)) * 0.1).astype(np.float32)
    out = kernel(x, wq, wk, wv)
    print("out", out.shape, out.dtype, float(np.abs(out).max()))



# revision 23
# speedup vs baseline: 1.0113x; 1.0113x over previous
"""AttentionConv Trainium2 kernel (8 NeuronCores, data-parallel over batch).

Reference math (per batch b, channel o, position (h,w), 7x7 window d=(di,dj)):
    q = wq @ x, k = wk @ x, v = wv @ x   (1x1 convs, channel matmuls)
    logits_d = q * k_d + rel             (k_d = zero-padded k shifted by d)
    out = sum_d softmax_d(logits) * v_d
`rel` is constant along the softmax axis, so it cancels. With zero padding,
out-of-bounds taps contribute exp(0)=1 to the denominator and 0 to the
numerator, which the zero-padded k/v slabs reproduce exactly.

v3 schedule (HW-measured: DVE bf16 tensor_tensor ~0.53ns/elem in 2x mode,
+3% for odd-offset windows; ACT 1/1.2GHz/elem; PE warm MM cadence 216ns at
N=512 with per-MM LDWEIGHTS overlapped by the reorder window):
  7 dj-groups; per group ONE mega t-mul t7[7,32,64] = bcast(q) * kwin4D
  (stride-0 q + overlapping [72,72,1] window AP), ONE 14336-elem exp, ONE
  mega u-mul, then 56 identity matmuls accumulating den/num in PSUM.
  DVE paces at ~15.3us/group; exp (12.2) and PE (12.1) hide under it.
  Tail: DVE reciprocal_approx_fast (saves the ACT ln/exp table + passes).
Partition layout: p = g*64 + o for H-halves g in {0,1}; free dim = (32,64).
GPSIMD only does head border memsets (its SBUF port contends with DVE).
"""

import numpy as np
import ml_dtypes

import concourse.bass as bass
import concourse.tile as tile
from concourse import bacc, mybir
from concourse.bass_utils import run_bass_kernel_spmd

N_CORES = 8
B, C, H, W, O = 8, 64, 64, 64, 64
KS, PAD = 7, 3
HW = H * W                      # 4096
HG = H // 2                     # 32 rows per partition group
NHALF = HG * W                  # 2048 free elements per partition
RSLAB = HG + KS - 1             # 38 padded slab rows
LPAD = 4                        # left col pad
CSLAB = W + LPAD + PAD + 1      # 72 cols (even row stride)
SLAB = RSLAB * CSLAB            # 2736

F32 = mybir.dt.float32
BF16 = mybir.dt.bfloat16
_NPBF16 = ml_dtypes.bfloat16

CFG = {
    "tbufs": 2, "ebufs": 2, "ubufs": 2,
    "t_split": 2,                # t-mul mega-ops per group
    "u_split": 2,                # u-mul mega-ops per group (earlier PE start)
    "u_split_last": 4,           # quarters on the last group: shorter PE drain
    "exp_split": 2,              # ACTIVATEs per group
    # odd dj first: those windows are 4B-aligned in the base slabs, so the
    # 1-shifted twins (for even dj) can be built on ACT slack meanwhile
    "dj_order": (1, 3, 5, 0, 2, 4, 6),
}


def build_program():
    nc = bacc.Bacc("TRN2", target_bir_lowering=False, debug=False,
                   num_devices=N_CORES)

    x_d = nc.dram_tensor("x", [C, HW], BF16, kind="ExternalInput").ap()
    wqT_d = nc.dram_tensor("wqT", [C, O], BF16, kind="ExternalInput").ap()
    wkT_d = nc.dram_tensor("wkT", [C, O], BF16, kind="ExternalInput").ap()
    wvT_d = nc.dram_tensor("wvT", [C, O], BF16, kind="ExternalInput").ap()
    ident_d = nc.dram_tensor("ident", [128, 128], BF16, kind="ExternalInput").ap()
    out_d = nc.dram_tensor("out", [O, H, W], BF16, kind="ExternalOutput").ap()

    with tile.TileContext(nc) as tc:
        _build(tc, x_d, wqT_d, wkT_d, wvT_d, ident_d, out_d)

    nc.compile()
    return nc


def _win(slab_flat, c0, di0, ndi):
    """[128, ndi, HG, W] overlapping window view of a [128, SLAB] slab:
    dims (di, r, c) with steps (CSLAB, CSLAB, 1), rows di0.., column c0."""
    t3 = slab_flat.rearrange("p (r c) -> p r c", r=RSLAB)
    v = t3[:, di0:di0 + HG, c0:c0 + W].unsqueeze(1).broadcast_to(
        [128, ndi, HG, W])
    w = v.copy()
    a = w.ap
    a[1] = [CSLAB, ndi]
    w.ap = a
    return w


def _build(tc, x_d, wqT_d, wkT_d, wvT_d, ident_d, out_d):
    nc = tc.nc
    from contextlib import ExitStack

    with ExitStack() as ctx:
        konst = ctx.enter_context(tc.tile_pool(name="konst", bufs=1))
        big = ctx.enter_context(tc.tile_pool(name="big", bufs=1))

        # --- inputs to SBUF ---
        # DMA issues serialize ~2.5us each per queue: wkT leads the sync
        # queue (tiny, unblocks PE warmup), then x ships as one half per
        # HWDGE queue into the same xb tile. ident rides Pool's SWDGE
        # after the memsets (first needed at the accum matmuls ~16us in).
        # xb lives in the long-lived pool: reusing its SBUF for the loop
        # pools would make the first t-mul wait on the v projection's last
        # xb read (measured +2.3us).
        wkT_sb = konst.tile([C, O], BF16, name="wkT_sb")
        nc.sync.dma_start(wkT_sb[:], wkT_d[:])
        xb = big.tile([C, HW], BF16, name="xb")
        nc.sync.dma_start(xb[:, 0:HW // 2], x_d[:, 0:HW // 2])
        nc.scalar.dma_start(xb[:, HW // 2:HW], x_d[:, HW // 2:HW])
        wqT_sb = konst.tile([C, O], BF16, name="wqT_sb")
        nc.sync.dma_start(wqT_sb[:], wqT_d[:])
        ident_sb = konst.tile([128, 128], BF16, name="ident_sb")
        wvT_sb = konst.tile([C, O], BF16, name="wvT_sb")
        nc.scalar.dma_start(wvT_sb[:], wvT_d[:])

        # --- padded k/v slabs (zeroed borders) + q ---
        # (no shifted twins: their SBUF space is what xb now keeps; the
        # odd-offset window penalty is ~3% on the 4 even-dj groups)
        q_sb = big.tile([128, HG, W], BF16, name="q_sb")
        kp = big.tile([128, SLAB], BF16, name="kp")
        vp = big.tile([128, SLAB], BF16, name="vp")
        kp3 = kp.rearrange("p (r c) -> p r c", r=RSLAB)
        vp3 = vp.rearrange("p (r c) -> p r c", r=RSLAB)
        # GPSIMD memsets run first (no deps) — off DVE's port before the loop
        for t3 in (kp3, vp3):
            nc.gpsimd.memset(t3[:, 0:PAD, :], 0.0)
            nc.gpsimd.memset(t3[:, RSLAB - PAD:RSLAB, :], 0.0)
            nc.gpsimd.memset(t3[:, PAD:RSLAB - PAD, 0:LPAD], 0.0)
            nc.gpsimd.memset(t3[:, PAD:RSLAB - PAD, LPAD + W:CSLAB], 0.0)
        nc.gpsimd.dma_start(ident_sb[:], ident_d[:])

        proj_ctx = ExitStack()
        # bufs=6: with 4, the q-projection matmuls stalled ~1.5us on the
        # psum buffer rotation behind the k evacuations (measured)
        psum = proj_ctx.enter_context(
            tc.tile_pool(name="psum", bufs=6, space="PSUM"))

        # PE p-state warmup: dummy wkT@wkT matmuls keep PE busy from wkT
        # arrival (~9.3us) until x lands (~12.5us), so the projections run
        # at full clock (2.4GHz needs ~3us of continuous PE activity).
        warm = psum.tile([128, 512], F32, tag="proj", name="warm_ps")
        for wi in range(46):
            nc.tensor.matmul(warm[0:64, 0:64], wkT_sb[:], wkT_sb[:],
                             start=True, stop=True)

        # group 0 slab rows r hold image rows r-3 (valid r in [3,38));
        # group 1 slab rows r hold image rows r+29 (valid r in [0,35)).
        chunk_rows = [(0, 8), (8, 8), (16, 8), (24, 8), (32, 3)]

        def project_kv(wT_sb, dst3, name, evac_even, evac_odd):
            for ci, (r0, nr) in enumerate(chunk_rows):
                n = nr * W
                ps = psum.tile([128, 512], F32, tag="proj", name=f"{name}_ps{ci}")
                nc.tensor.matmul(ps[0:64, :n], wT_sb[:],
                                 xb[:, r0 * W:(r0 + nr) * W],
                                 start=True, stop=True)
                nc.tensor.matmul(ps[64:128, :n], wT_sb[:],
                                 xb[:, (29 + r0) * W:(29 + r0 + nr) * W],
                                 start=True, stop=True)
                src = ps[:, :n].rearrange("p (a b) -> p a b", a=nr)
                evac = evac_even if ci % 2 == 0 else evac_odd
                evac(dst3[0:64, 3 + r0:3 + r0 + nr, LPAD:LPAD + W], src[0:64])
                evac(dst3[64:128, r0:r0 + nr, LPAD:LPAD + W], src[64:128])

        # k first (the first t-mul needs it); evacs split DVE/ACT
        project_kv(wkT_sb, kp3, "k", nc.vector.tensor_copy, nc.scalar.copy)

        # q evacs alternate DVE/ACT: they gate the first t-mul
        for cchunk in range(4):
            ps = psum.tile([128, 512], F32, tag="proj", name=f"q_ps{cchunk}")
            for g in (0, 1):
                rhs = xb[:, g * NHALF + cchunk * 512: g * NHALF + (cchunk + 1) * 512]
                nc.tensor.matmul(ps[g * 64:(g + 1) * 64, :], wqT_sb[:], rhs,
                                 start=True, stop=True)
            evac = nc.vector.tensor_copy if cchunk % 2 == 0 else nc.scalar.copy
            evac(q_sb[:, cchunk * 8:(cchunk + 1) * 8, :],
                 ps[:].rearrange("p (a b) -> p a b", a=8))

        project_kv(wvT_sb, vp3, "v", nc.scalar.copy, nc.scalar.copy)

        proj_ctx.close()

        # --- main loop: 7 dj-groups x 7 di ---
        acc = ctx.enter_context(tc.tile_pool(name="acc", bufs=1, space="PSUM"))
        den_ps = acc.tile([128, NHALF], F32, name="den_ps")
        num_ps = acc.tile([128, NHALF], F32, name="num_ps")

        loop_ctx = ExitStack()
        tpool = loop_ctx.enter_context(tc.tile_pool(name="tpool", bufs=CFG["tbufs"]))
        epool = loop_ctx.enter_context(tc.tile_pool(name="epool", bufs=CFG["ebufs"]))
        upool = loop_ctx.enter_context(tc.tile_pool(name="upool", bufs=CFG["ubufs"]))

        qb = q_sb[:].unsqueeze(1).broadcast_to([128, KS, HG, W])

        n_off = KS * KS
        for gi, dj in enumerate(CFG["dj_order"]):
            c0 = dj + LPAD - PAD
            kb, kc = kp, c0
            vb, vc = vp, c0
            # last group: fine-grained interleaved t/exp/u pieces shorten
            # the end drain (the t->exp->u chain of the final di slices);
            # measured ~3us of DVE idle there with the coarse 2/2/4 splits
            last_group = gi == KS - 1
            tb = (0, 2, 4, 6, 7) if last_group else (0, 3, 7)
            eb = tb
            ub = (0, 2, 4, 6, 7) if last_group else (0, 3, 7)
            t7 = tpool.tile([128, KS, HG, W], BF16, tag="t", name=f"t7_{gi}")
            for si in range(len(tb) - 1):
                lo, hi = tb[si], tb[si + 1]
                nc.vector.tensor_mul(t7[:, lo:hi], qb[:, lo:hi],
                                     _win(kb, kc, lo, hi - lo))
            e7 = epool.tile([128, KS, HG, W], BF16, tag="e", name=f"e7_{gi}")
            t7f = t7.rearrange("p a b c -> p (a b c)")
            e7f = e7.rearrange("p a b c -> p (a b c)")
            for si in range(len(eb) - 1):
                lo, hi = eb[si] * NHALF, eb[si + 1] * NHALF
                nc.scalar.activation(e7f[:, lo:hi], t7f[:, lo:hi],
                                     mybir.ActivationFunctionType.Exp)
            u7 = upool.tile([128, KS, HG, W], BF16, tag="u", name=f"u7_{gi}")
            for si in range(len(ub) - 1):
                lo, hi = ub[si], ub[si + 1]
                nc.vector.tensor_mul(u7[:, lo:hi], e7[:, lo:hi],
                                     _win(vb, vc, lo, hi - lo))
            # PE accumulation, phase-ordered per exp/u piece: den matmuls
            # (gated only by the exp piece) never queue behind num matmuls
            # (gated by the later u piece) in PE's in-order queue. This
            # keeps PE flowing and lets the tail recip start right after
            # the final den piece instead of ~2.3us later (measured).
            for si in range(len(ub) - 1):
                plo, phi = ub[si], ub[si + 1]
                for di in range(plo, phi):
                    for cc in range(4):
                        nc.tensor.matmul(
                            den_ps[:, cc * 512:(cc + 1) * 512], ident_sb[:],
                            e7[:, di, cc * 8:(cc + 1) * 8, :],
                            start=(gi == 0 and di == 0),
                            stop=(gi == KS - 1 and di == KS - 1),
                            skip_group_check=True)
                for di in range(plo, phi):
                    for cc in range(4):
                        nc.tensor.matmul(
                            num_ps[:, cc * 512:(cc + 1) * 512], ident_sb[:],
                            u7[:, di, cc * 8:(cc + 1) * 8, :],
                            start=(gi == 0 and di == 0),
                            stop=(gi == KS - 1 and di == KS - 1),
                            skip_group_check=True)

        loop_ctx.close()

        # --- divide and store (DVE approx reciprocal; den>0, well-scaled) ---
        tail_pool = ctx.enter_context(tc.tile_pool(name="tail", bufs=1))
        den_r = tail_pool.tile([128, NHALF], F32, name="den_r")
        out_sb = tail_pool.tile([128, NHALF], BF16, name="out_sb")
        out3 = out_sb.rearrange("p (a b) -> p a b", a=HG)
        for cc in range(4):
            sl = slice(cc * 512, (cc + 1) * 512)
            nc.vector.reciprocal_approx_fast(den_r[:, sl], den_ps[:, sl])
            nc.vector.tensor_mul(out_sb[:, sl], num_ps[:, sl], den_r[:, sl])
            rsl = slice(cc * 8, (cc + 1) * 8)
            q = nc.sync if cc % 2 == 0 else nc.scalar
            q.dma_start(out_d[:, rsl, :], out3[0:64, rsl, :])
            q.dma_start(out_d[:, HG + cc * 8:HG + (cc + 1) * 8, :],
                        out3[64:128, rsl, :])


_NC_CACHE = None


def _get_nc():
    global _NC_CACHE
    if _NC_CACHE is None:
        _NC_CACHE = build_program()
    return _NC_CACHE


def prepare_in_maps(x, wq, wk, wv):
    x = np.ascontiguousarray(np.asarray(x, np.float32).astype(_NPBF16))
    wqT = np.ascontiguousarray(np.asarray(wq, np.float32).T.astype(_NPBF16))
    wkT = np.ascontiguousarray(np.asarray(wk, np.float32).T.astype(_NPBF16))
    wvT = np.ascontiguousarray(np.asarray(wv, np.float32).T.astype(_NPBF16))
    ident = np.eye(128, dtype=_NPBF16)
    return [
        {"x": x[i].reshape(C, HW), "wqT": wqT, "wkT": wkT, "wvT": wvT,
         "ident": ident}
        for i in range(x.shape[0])
    ]


def run(in_maps, **kw):
    nc = _get_nc()
    return run_bass_kernel_spmd(nc, in_maps, list(range(N_CORES)), **kw)


def kernel(x, wq, wk, wv, rel_w=None, rel_h=None, kernel_size=7, padding=3,
           **_ignored):
    # rel_w/rel_h are constant along the softmax axis, so they cancel.
    assert int(kernel_size) == KS and int(padding) == PAD
    res = run(prepare_in_maps(x, wq, wk, wv))
    out = np.stack([res.results[i]["out"] for i in range(N_CORES)], axis=0)
    return out.astype(np.float32)


if __name__ == "__main__":
    rng = np.random.default_rng(0)
    x = rng.standard_normal((B, C, H, W), dtype=np.float32)
    wq = (rng.standard_normal((O, C)) * 0.1).astype(np.float32)
    wk = (rng.standard_normal((O, C)) * 0.1).astype(np.float32)
    wv = (rng.standard_normal((O, C)) * 0.1).astype(np.float32)
    out = kernel(x, wq, wk, wv)
    print("out", out.shape, out.dtype, float(np.abs(out).max()))



# revision 24
# speedup vs baseline: 1.0156x; 1.0043x over previous
"""AttentionConv Trainium2 kernel (8 NeuronCores, data-parallel over batch).

Reference math (per batch b, channel o, position (h,w), 7x7 window d=(di,dj)):
    q = wq @ x, k = wk @ x, v = wv @ x   (1x1 convs, channel matmuls)
    logits_d = q * k_d + rel             (k_d = zero-padded k shifted by d)
    out = sum_d softmax_d(logits) * v_d
`rel` is constant along the softmax axis, so it cancels. With zero padding,
out-of-bounds taps contribute exp(0)=1 to the denominator and 0 to the
numerator, which the zero-padded k/v slabs reproduce exactly.

v3 schedule (HW-measured: DVE bf16 tensor_tensor ~0.53ns/elem in 2x mode,
+3% for odd-offset windows; ACT 1/1.2GHz/elem; PE warm MM cadence 216ns at
N=512 with per-MM LDWEIGHTS overlapped by the reorder window):
  7 dj-groups; per group ONE mega t-mul t7[7,32,64] = bcast(q) * kwin4D
  (stride-0 q + overlapping [72,72,1] window AP), ONE 14336-elem exp, ONE
  mega u-mul, then 56 identity matmuls accumulating den/num in PSUM.
  DVE paces at ~15.3us/group; exp (12.2) and PE (12.1) hide under it.
  Tail: DVE reciprocal_approx_fast (saves the ACT ln/exp table + passes).
Partition layout: p = g*64 + o for H-halves g in {0,1}; free dim = (32,64).
GPSIMD only does head border memsets (its SBUF port contends with DVE).
"""

import numpy as np
import ml_dtypes

import concourse.bass as bass
import concourse.tile as tile
from concourse import bacc, mybir
from concourse.bass_utils import run_bass_kernel_spmd

N_CORES = 8
B, C, H, W, O = 8, 64, 64, 64, 64
KS, PAD = 7, 3
HW = H * W                      # 4096
HG = H // 2                     # 32 rows per partition group
NHALF = HG * W                  # 2048 free elements per partition
RSLAB = HG + KS - 1             # 38 padded slab rows
LPAD = 4                        # left col pad
CSLAB = W + LPAD + PAD + 1      # 72 cols (even row stride)
SLAB = RSLAB * CSLAB            # 2736

F32 = mybir.dt.float32
BF16 = mybir.dt.bfloat16
_NPBF16 = ml_dtypes.bfloat16

CFG = {
    "tbufs": 2, "ebufs": 2, "ubufs": 2,
    "t_split": 2,                # t-mul mega-ops per group
    "u_split": 2,                # u-mul mega-ops per group (earlier PE start)
    "u_split_last": 4,           # quarters on the last group: shorter PE drain
    "exp_split": 2,              # ACTIVATEs per group
    # odd dj first: those windows are 4B-aligned in the base slabs, so the
    # 1-shifted twins (for even dj) can be built on ACT slack meanwhile
    "dj_order": (1, 3, 5, 0, 2, 4, 6),
}


def build_program():
    nc = bacc.Bacc("TRN2", target_bir_lowering=False, debug=False,
                   num_devices=N_CORES)

    x_d = nc.dram_tensor("x", [C, HW], BF16, kind="ExternalInput").ap()
    wqT_d = nc.dram_tensor("wqT", [C, O], BF16, kind="ExternalInput").ap()
    wkT_d = nc.dram_tensor("wkT", [C, O], BF16, kind="ExternalInput").ap()
    wvT_d = nc.dram_tensor("wvT", [C, O], BF16, kind="ExternalInput").ap()
    ident_d = nc.dram_tensor("ident", [128, 128], BF16, kind="ExternalInput").ap()
    out_d = nc.dram_tensor("out", [O, H, W], BF16, kind="ExternalOutput").ap()

    with tile.TileContext(nc) as tc:
        _build(tc, x_d, wqT_d, wkT_d, wvT_d, ident_d, out_d)

    nc.compile()
    return nc


def _win(slab_flat, c0, di0, ndi):
    """[128, ndi, HG, W] overlapping window view of a [128, SLAB] slab:
    dims (di, r, c) with steps (CSLAB, CSLAB, 1), rows di0.., column c0."""
    t3 = slab_flat.rearrange("p (r c) -> p r c", r=RSLAB)
    v = t3[:, di0:di0 + HG, c0:c0 + W].unsqueeze(1).broadcast_to(
        [128, ndi, HG, W])
    w = v.copy()
    a = w.ap
    a[1] = [CSLAB, ndi]
    w.ap = a
    return w


def _build(tc, x_d, wqT_d, wkT_d, wvT_d, ident_d, out_d):
    nc = tc.nc
    from contextlib import ExitStack

    with ExitStack() as ctx:
        konst = ctx.enter_context(tc.tile_pool(name="konst", bufs=1))
        big = ctx.enter_context(tc.tile_pool(name="big", bufs=1))

        # --- inputs to SBUF ---
        # DMA issues serialize ~2.5us each per queue: wkT leads the sync
        # queue (tiny, unblocks PE warmup), then x ships as one half per
        # HWDGE queue into the same xb tile. ident rides Pool's SWDGE
        # after the memsets (first needed at the accum matmuls ~16us in).
        # xb lives in the long-lived pool: reusing its SBUF for the loop
        # pools would make the first t-mul wait on the v projection's last
        # xb read (measured +2.3us).
        wkT_sb = konst.tile([C, O], BF16, name="wkT_sb")
        nc.sync.dma_start(wkT_sb[:], wkT_d[:])
        xb = big.tile([C, HW], BF16, name="xb")
        nc.sync.dma_start(xb[:, 0:HW // 2], x_d[:, 0:HW // 2])
        nc.scalar.dma_start(xb[:, HW // 2:HW], x_d[:, HW // 2:HW])
        wqT_sb = konst.tile([C, O], BF16, name="wqT_sb")
        nc.sync.dma_start(wqT_sb[:], wqT_d[:])
        ident_sb = konst.tile([128, 128], BF16, name="ident_sb")
        wvT_sb = konst.tile([C, O], BF16, name="wvT_sb")
        nc.scalar.dma_start(wvT_sb[:], wvT_d[:])

        # --- padded k/v slabs (zeroed borders) + q ---
        # (no shifted twins: their SBUF space is what xb now keeps; the
        # odd-offset window penalty is ~3% on the 4 even-dj groups)
        q_sb = big.tile([128, HG, W], BF16, name="q_sb")
        kp = big.tile([128, SLAB], BF16, name="kp")
        vp = big.tile([128, SLAB], BF16, name="vp")
        kp3 = kp.rearrange("p (r c) -> p r c", r=RSLAB)
        vp3 = vp.rearrange("p (r c) -> p r c", r=RSLAB)
        # GPSIMD memsets run first (no deps) — off DVE's port before the loop
        for t3 in (kp3, vp3):
            nc.gpsimd.memset(t3[:, 0:PAD, :], 0.0)
            nc.gpsimd.memset(t3[:, RSLAB - PAD:RSLAB, :], 0.0)
            nc.gpsimd.memset(t3[:, PAD:RSLAB - PAD, 0:LPAD], 0.0)
            nc.gpsimd.memset(t3[:, PAD:RSLAB - PAD, LPAD + W:CSLAB], 0.0)
        nc.gpsimd.dma_start(ident_sb[:], ident_d[:])

        proj_ctx = ExitStack()
        # bufs=6: with 4, the q-projection matmuls stalled ~1.5us on the
        # psum buffer rotation behind the k evacuations (measured)
        psum = proj_ctx.enter_context(
            tc.tile_pool(name="psum", bufs=6, space="PSUM"))

        # PE p-state warmup: dummy wkT@wkT matmuls keep PE busy from wkT
        # arrival (~9.3us) until x lands (~12.5us), so the projections run
        # at full clock (2.4GHz needs ~3us of continuous PE activity).
        warm = psum.tile([128, 512], F32, tag="proj", name="warm_ps")
        for wi in range(46):
            nc.tensor.matmul(warm[0:64, 0:64], wkT_sb[:], wkT_sb[:],
                             start=True, stop=True)

        # group 0 slab rows r hold image rows r-3 (valid r in [3,38));
        # group 1 slab rows r hold image rows r+29 (valid r in [0,35)).
        chunk_rows = [(0, 8), (8, 8), (16, 8), (24, 8), (32, 3)]

        def project_kv(wT_sb, dst3, name, evac_even, evac_odd):
            for ci, (r0, nr) in enumerate(chunk_rows):
                n = nr * W
                ps = psum.tile([128, 512], F32, tag="proj", name=f"{name}_ps{ci}")
                nc.tensor.matmul(ps[0:64, :n], wT_sb[:],
                                 xb[:, r0 * W:(r0 + nr) * W],
                                 start=True, stop=True)
                nc.tensor.matmul(ps[64:128, :n], wT_sb[:],
                                 xb[:, (29 + r0) * W:(29 + r0 + nr) * W],
                                 start=True, stop=True)
                src = ps[:, :n].rearrange("p (a b) -> p a b", a=nr)
                evac = evac_even if ci % 2 == 0 else evac_odd
                evac(dst3[0:64, 3 + r0:3 + r0 + nr, LPAD:LPAD + W], src[0:64])
                evac(dst3[64:128, r0:r0 + nr, LPAD:LPAD + W], src[64:128])

        # k first (the first t-mul needs it); evacs split DVE/ACT
        project_kv(wkT_sb, kp3, "k", nc.vector.tensor_copy, nc.scalar.copy)

        # q evacs alternate DVE/ACT: they gate the first t-mul
        for cchunk in range(4):
            ps = psum.tile([128, 512], F32, tag="proj", name=f"q_ps{cchunk}")
            for g in (0, 1):
                rhs = xb[:, g * NHALF + cchunk * 512: g * NHALF + (cchunk + 1) * 512]
                nc.tensor.matmul(ps[g * 64:(g + 1) * 64, :], wqT_sb[:], rhs,
                                 start=True, stop=True)
            evac = nc.vector.tensor_copy if cchunk % 2 == 0 else nc.scalar.copy
            evac(q_sb[:, cchunk * 8:(cchunk + 1) * 8, :],
                 ps[:].rearrange("p (a b) -> p a b", a=8))

        project_kv(wvT_sb, vp3, "v", nc.scalar.copy, nc.scalar.copy)

        proj_ctx.close()

        # --- main loop: 7 dj-groups x 7 di ---
        acc = ctx.enter_context(tc.tile_pool(name="acc", bufs=1, space="PSUM"))
        den_ps = acc.tile([128, NHALF], F32, name="den_ps")
        num_ps = acc.tile([128, NHALF], F32, name="num_ps")

        loop_ctx = ExitStack()
        tpool = loop_ctx.enter_context(tc.tile_pool(name="tpool", bufs=CFG["tbufs"]))
        epool = loop_ctx.enter_context(tc.tile_pool(name="epool", bufs=CFG["ebufs"]))
        upool = loop_ctx.enter_context(tc.tile_pool(name="upool", bufs=CFG["ubufs"]))

        qb = q_sb[:].unsqueeze(1).broadcast_to([128, KS, HG, W])

        n_off = KS * KS
        for gi, dj in enumerate(CFG["dj_order"]):
            c0 = dj + LPAD - PAD
            kb, kc = kp, c0
            vb, vc = vp, c0
            # last group: fine-grained interleaved t/exp/u pieces shorten
            # the end drain (the t->exp->u chain of the final di slices);
            # measured ~3us of DVE idle there with the coarse 2/2/4 splits
            last_group = gi == KS - 1
            tb = (0, 2, 4, 6, 7) if last_group else (0, 3, 7)
            eb = tb
            ub = (0, 2, 4, 6, 7) if last_group else (0, 3, 7)
            t7 = tpool.tile([128, KS, HG, W], BF16, tag="t", name=f"t7_{gi}")
            for si in range(len(tb) - 1):
                lo, hi = tb[si], tb[si + 1]
                nc.vector.tensor_mul(t7[:, lo:hi], qb[:, lo:hi],
                                     _win(kb, kc, lo, hi - lo))
            e7 = epool.tile([128, KS, HG, W], BF16, tag="e", name=f"e7_{gi}")
            t7f = t7.rearrange("p a b c -> p (a b c)")
            e7f = e7.rearrange("p a b c -> p (a b c)")
            for si in range(len(eb) - 1):
                lo, hi = eb[si] * NHALF, eb[si + 1] * NHALF
                nc.scalar.activation(e7f[:, lo:hi], t7f[:, lo:hi],
                                     mybir.ActivationFunctionType.Exp)
            u7 = upool.tile([128, KS, HG, W], BF16, tag="u", name=f"u7_{gi}")
            for si in range(len(ub) - 1):
                lo, hi = ub[si], ub[si + 1]
                nc.vector.tensor_mul(u7[:, lo:hi], e7[:, lo:hi],
                                     _win(vb, vc, lo, hi - lo))
            # PE accumulation, phase-ordered per exp/u piece: den matmuls
            # (gated only by the exp piece) never queue behind num matmuls
            # (gated by the later u piece) in PE's in-order queue. This
            # keeps PE flowing and lets the tail recip start right after
            # the final den piece instead of ~2.3us later (measured).
            for si in range(len(ub) - 1):
                plo, phi = ub[si], ub[si + 1]
                for di in range(plo, phi):
                    for cc in range(4):
                        nc.tensor.matmul(
                            den_ps[:, cc * 512:(cc + 1) * 512], ident_sb[:],
                            e7[:, di, cc * 8:(cc + 1) * 8, :],
                            start=(gi == 0 and di == 0),
                            stop=(gi == KS - 1 and di == KS - 1),
                            skip_group_check=True)
                for di in range(plo, phi):
                    for cc in range(4):
                        nc.tensor.matmul(
                            num_ps[:, cc * 512:(cc + 1) * 512], ident_sb[:],
                            u7[:, di, cc * 8:(cc + 1) * 8, :],
                            start=(gi == 0 and di == 0),
                            stop=(gi == KS - 1 and di == KS - 1),
                            skip_group_check=True)

        loop_ctx.close()

        # --- divide and store (DVE approx reciprocal; den>0, well-scaled) ---
        tail_pool = ctx.enter_context(tc.tile_pool(name="tail", bufs=1))
        den_r = tail_pool.tile([128, NHALF], F32, name="den_r")
        out_sb = tail_pool.tile([128, NHALF], BF16, name="out_sb")
        out3 = out_sb.rearrange("p (a b) -> p a b", a=HG)
        # all recips first: they are den-gated (ready before num), so they
        # must not queue behind the num-gated muls in DVE's in-order queue
        for cc in range(4):
            sl = slice(cc * 512, (cc + 1) * 512)
            nc.vector.reciprocal_approx_fast(den_r[:, sl], den_ps[:, sl])
        for cc in range(4):
            sl = slice(cc * 512, (cc + 1) * 512)
            nc.vector.tensor_mul(out_sb[:, sl], num_ps[:, sl], den_r[:, sl])
            rsl = slice(cc * 8, (cc + 1) * 8)
            q = nc.sync if cc % 2 == 0 else nc.scalar
            q.dma_start(out_d[:, rsl, :], out3[0:64, rsl, :])
            q.dma_start(out_d[:, HG + cc * 8:HG + (cc + 1) * 8, :],
                        out3[64:128, rsl, :])


_NC_CACHE = None


def _get_nc():
    global _NC_CACHE
    if _NC_CACHE is None:
        _NC_CACHE = build_program()
    return _NC_CACHE


def prepare_in_maps(x, wq, wk, wv):
    x = np.ascontiguousarray(np.asarray(x, np.float32).astype(_NPBF16))
    wqT = np.ascontiguousarray(np.asarray(wq, np.float32).T.astype(_NPBF16))
    wkT = np.ascontiguousarray(np.asarray(wk, np.float32).T.astype(_NPBF16))
    wvT = np.ascontiguousarray(np.asarray(wv, np.float32).T.astype(_NPBF16))
    ident = np.eye(128, dtype=_NPBF16)
    return [
        {"x": x[i].reshape(C, HW), "wqT": wqT, "wkT": wkT, "wvT": wvT,
         "ident": ident}
        for i in range(x.shape[0])
    ]


def run(in_maps, **kw):
    nc = _get_nc()
    return run_bass_kernel_spmd(nc, in_maps, list(range(N_CORES)), **kw)


def kernel(x, wq, wk, wv, rel_w=None, rel_h=None, kernel_size=7, padding=3,
           **_ignored):
    # rel_w/rel_h are constant along the softmax axis, so they cancel.
    assert int(kernel_size) == KS and int(padding) == PAD
    res = run(prepare_in_maps(x, wq, wk, wv))
    out = np.stack([res.results[i]["out"] for i in range(N_CORES)], axis=0)
    return out.astype(np.float32)


if __name__ == "__main__":
    rng = np.random.default_rng(0)
    x = rng.standard_normal((B, C, H, W), dtype=np.float32)
    wq = (rng.standard_normal((O, C)) * 0.1).astype(np.float32)
    wk = (rng.standard_normal((O, C)) * 0.1).astype(np.float32)
    wv = (rng.standard_normal((O, C)) * 0.1).astype(np.float32)
    out = kernel(x, wq, wk, wv)
    print("out", out.shape, out.dtype, float(np.abs(out).max()))

